# revision 6
# baseline (speedup 1.0000x reference)
"""AllDeepSet hypergraph GNN on 8 TRN2 NeuronCores.

Strategy:
  - Nodes sharded 12500/core (contiguous ranges, all_batch is sorted so the
    readout is shard-local). Incidences sharded by src ownership.
  - Per layer: node MLP (feature-major bf16 matmuls) -> write node-major h
    table to HBM -> dma_gather h[src] in dst-sorted order -> one-hot matmul
    scatter into 128-edge PSUM windows -> bf16 AllReduce of the [128, MP]
    edge partials -> edge MLPs -> write e table -> dma_gather e[dst] in
    src-sorted order -> one-hot matmul scatter into 128-node windows ->
    node MLP.
  - Readout: per-core G matrix (one-hot(graph)/count) matmul against
    node-major tiles, AllReduce [64,128], classifier MLP on every core.
  - All host-side index prep (sorting, window padding, int16 wrapping) is
    done in numpy inside kernel().
"""

import numpy as np
import ml_dtypes

import concourse.bass as bass
import concourse.bacc as bacc
import concourse.tile as tile
import concourse.mybir as mybir
from concourse.bass_utils import run_bass_kernel_spmd
from concourse.masks import make_identity

BF16 = ml_dtypes.bfloat16
NCORES = 8
D = 128
GATHER_CHUNK = 1024  # idxs per dma_gather call (SWDGE ring limit)

_ROLES = ["ve_enc", "ve_dec", "ev_enc", "ev_dec"]


def _wrap16(a):
    """dma_gather index layout: [128, n/16] int16, idx i at [16r + i%16, i//16]."""
    return np.tile(a.reshape(-1, 16).T, (NCORES, 1)).copy()


def _wrap128(a, nt):
    """per-incidence metadata layout: [128, NT], incidence t*128+p at [p, t]."""
    return np.ascontiguousarray(a.reshape(nt, 128).T)


def _preprocess(inputs, N, M, E, G, L):
    NS = N // NCORES
    NSP = -(-NS // 128) * 128
    NW2 = NSP // 128
    MP = -(-M // 512) * 512
    NW1 = MP // 128

    src = np.asarray(inputs["v2e_src"]).astype(np.int64)
    dst = np.asarray(inputs["v2e_dst"]).astype(np.int64)
    batch = np.asarray(inputs["all_batch"]).astype(np.int64)

    per_core = []
    cnt1 = np.zeros((NCORES, NW1), np.int64)
    cnt2 = np.zeros((NCORES, NW2), np.int64)
    for c in range(NCORES):
        m = (src >= c * NS) & (src < (c + 1) * NS)
        sl = src[m] - c * NS
        dg = dst[m]
        o1 = np.argsort(dg, kind="stable")
        sl1, dg1 = sl[o1], dg[o1]
        w1 = dg1 >> 7
        cnt1[c] = np.bincount(w1, minlength=NW1)
        o2 = np.argsort(sl, kind="stable")
        sl2, dg2 = sl[o2], dg[o2]
        w2 = sl2 >> 7
        cnt2[c] = np.bincount(w2, minlength=NW2)
        per_core.append((sl1, dg1, w1, sl2, dg2, w2))

    def tiles_of(cnt):
        return -(-cnt.max(axis=0) // 128)  # per-window tile count, shared by all cores

    T1 = tiles_of(cnt1)
    T1[-1] += (-T1.sum()) % 8
    NT1 = int(T1.sum())
    T2 = tiles_of(cnt2)
    T2[-1] += (-T2.sum()) % 8
    NT2 = int(T2.sum())
    base1 = np.concatenate([[0], np.cumsum(T1)])
    base2 = np.concatenate([[0], np.cumsum(T2)])

    cnt_g = np.bincount(batch, minlength=G).astype(np.float32)
    inv_cnt = 1.0 / np.maximum(cnt_g, 1.0)

    # weights / biases packing
    wts = np.zeros((128, 18 * 128), BF16)
    bias = np.zeros((128, 18), np.float32)
    col = 0

    def put_w(w):
        nonlocal col
        w = np.asarray(w, np.float32)
        wts[:, col * 128: col * 128 + w.shape[1]] = w.astype(BF16)
        col += 1

    bcol = 0

    def put_b(b):
        nonlocal bcol
        b = np.asarray(b, np.float32)
        bias[: b.shape[0], bcol] = b
        bcol += 1

    for role in _ROLES:
        for l in range(L):
            put_w(inputs[role + "_W1"][l]); put_w(inputs[role + "_W2"][l])
            put_b(inputs[role + "_b1"][l]); put_b(inputs[role + "_b2"][l])
    put_w(inputs["cls_W1"]); put_w(inputs["cls_W2"])
    put_b(inputs["cls_b1"]); put_b(inputs["cls_b2"])

    iota8 = np.tile(np.arange(128, dtype=np.float32), (128, 8)).astype(BF16)

    X = np.asarray(inputs["X"], np.float32)
    in_maps = []
    for c in range(NCORES):
        sl1, dg1, w1, sl2, dg2, w2 = per_core[c]

        def stream(vals_idx, vals_loc, w, base, nt):
            gidx = np.zeros(nt * 128, np.int16)
            loc = np.full(nt * 128, 300.0, np.float32)
            nw = len(base) - 1
            starts = np.concatenate([[0], np.cumsum(np.bincount(w, minlength=nw))])
            rank = np.arange(len(w)) - starts[w]
            pos = base[w] * 128 + rank
            gidx[pos] = vals_idx
            loc[pos] = vals_loc
            return _wrap16(gidx), _wrap128(loc.astype(BF16), nt)

        g1, l1 = stream(sl1, dg1 - (w1 << 7), w1, base1, NT1)
        g2, l2 = stream(dg2, sl2 - (w2 << 7), w2, base2, NT2)

        xf = np.zeros((128, NSP), BF16)
        xf[:, :NS] = X[c * NS:(c + 1) * NS].T.astype(BF16)

        gm = np.zeros((128, NW2 * 64), BF16)
        b = batch[c * NS:(c + 1) * NS]
        gmat = np.zeros((NSP, G), np.float32)
        gmat[np.arange(NS), b] = inv_cnt[b]
        for w in range(NW2):
            gm[:, w * 64:w * 64 + G] = gmat[w * 128:(w + 1) * 128, :].astype(BF16)

        b2row = np.zeros((64, 64), np.float32)
        b2row[:, :40] = np.asarray(inputs["cls_b2"], np.float32)[None, :]
        in_maps.append({
            "xfm": xf, "wts": wts, "bias": bias, "iota8": iota8,
            "gidx1": g1, "dloc1": l1, "gidx2": g2, "nloc2": l2, "gmat": gm,
            "b2row": b2row,
        })

    cfg = dict(N=N, M=M, E=E, G=G, L=L, NS=NS, NSP=NSP, MP=MP, NW1=NW1,
               NW2=NW2, T1=T1.tolist(), T2=T2.tolist(), NT1=NT1, NT2=NT2)
    return in_maps, cfg


def _build(cfg):
    NSP, MP = cfg["NSP"], cfg["MP"]
    NW1, NW2 = cfg["NW1"], cfg["NW2"]
    T1, T2 = cfg["T1"], cfg["T2"]
    NT1, NT2 = cfg["NT1"], cfg["NT2"]
    G, L = cfg["G"], cfg["L"]
    f32, bf16, i16 = mybir.dt.float32, mybir.dt.bfloat16, mybir.dt.int16
    RELU = mybir.ActivationFunctionType.Relu
    COPY = mybir.ActivationFunctionType.Copy
    EQ = mybir.AluOpType.is_equal

    nc = bacc.Bacc("TRN2", target_bir_lowering=False, debug=False,
                   num_devices=NCORES, num_swdge_queues=4)

    xfm_in = nc.dram_tensor("xfm", [128, NSP], bf16, kind="ExternalInput")
    wts_in = nc.dram_tensor("wts", [128, 18 * 128], bf16, kind="ExternalInput")
    bias_in = nc.dram_tensor("bias", [128, 18], f32, kind="ExternalInput")
    iota_in = nc.dram_tensor("iota8", [128, 8 * 128], bf16, kind="ExternalInput")
    g1_in = nc.dram_tensor("gidx1", [128, NT1 * 8], i16, kind="ExternalInput")
    l1_in = nc.dram_tensor("dloc1", [128, NT1], bf16, kind="ExternalInput")
    g2_in = nc.dram_tensor("gidx2", [128, NT2 * 8], i16, kind="ExternalInput")
    l2_in = nc.dram_tensor("nloc2", [128, NT2], bf16, kind="ExternalInput")
    gm_in = nc.dram_tensor("gmat", [128, NW2 * 64], bf16, kind="ExternalInput")
    b2r_in = nc.dram_tensor("b2row", [64, 64], f32, kind="ExternalInput")
    out = nc.dram_tensor("out", [G, 40], f32, kind="ExternalOutput")

    tbl_h = nc.dram_tensor("tbl_h", [NSP, 128], bf16, kind="Internal")
    tbl_e = nc.dram_tensor("tbl_e", [MP, 128], bf16, kind="Internal")

    # weight column index: roles x layers x (W1, W2), then cls
    def wslot(role, l, which):
        r = _ROLES.index(role)
        return (r * L + l) * 2 + (which - 1)

    def bslot(role, l, which):
        r = _ROLES.index(role)
        return (r * L + l) * 2 + (which - 1)

    with tile.TileContext(nc) as tc:
        with (
            tc.tile_pool(name="const", bufs=1) as cp,
            tc.tile_pool(name="pers", bufs=1) as pers,
            tc.tile_pool(name="gath", bufs=8) as gp,
            tc.tile_pool(name="oh", bufs=4) as ohp,
            tc.tile_pool(name="mlp", bufs=3) as mp_,
            tc.tile_pool(name="tpo", bufs=4) as tp,
            tc.tile_pool(name="psw", bufs=2, space="PSUM") as pp,
            tc.tile_pool(name="psm", bufs=3, space="PSUM") as ppm,
            tc.tile_pool(name="prr", bufs=1, space="PSUM") as prp,
            tc.tile_pool(name="pst", bufs=2, space="PSUM") as ppt,
            tc.tile_pool(name="dram", bufs=2, space="DRAM") as dram,
        ):
            # ---- load constants ----
            wts = cp.tile([128, 18 * 128], bf16)
            nc.sync.dma_start(wts[:], wts_in[:])
            bias = cp.tile([128, 18], f32)
            nc.sync.dma_start(bias[:], bias_in[:])
            iota8 = cp.tile([128, 8 * 128], bf16)
            nc.sync.dma_start(iota8[:], iota_in[:])
            gidx1 = cp.tile([128, NT1 * 8], i16)
            nc.sync.dma_start(gidx1[:], g1_in[:])
            dloc1 = cp.tile([128, NT1], bf16)
            nc.sync.dma_start(dloc1[:], l1_in[:])
            gidx2 = cp.tile([128, NT2 * 8], i16)
            nc.sync.dma_start(gidx2[:], g2_in[:])
            nloc2 = cp.tile([128, NT2], bf16)
            nc.sync.dma_start(nloc2[:], l2_in[:])
            gmat = cp.tile([128, NW2 * 64], bf16)
            nc.sync.dma_start(gmat[:], gm_in[:])
            b2row = cp.tile([64, 64], f32)
            nc.sync.dma_start(b2row[:], b2r_in[:])
            ident = cp.tile([128, 128], bf16)
            make_identity(nc, ident[:])

            node_fm = pers.tile([128, NSP], bf16)
            nc.sync.dma_start(node_fm[:], xfm_in[:])
            edge_acc = pers.tile([128, MP], bf16)

            def W(role, l, which):
                s = wslot(role, l, which)
                return wts[:, s * 128:(s + 1) * 128]

            def B(role, l, which):
                s = bslot(role, l, which)
                return bias[:, s:s + 1]

            def mlp_chunk(dst_ap, src_ap, w1, b1, w2, b2, cw):
                ps1 = ppm.tile([128, 512], f32, tag="psmlp")
                nc.tensor.matmul(out=ps1[:, :cw], lhsT=w1, rhs=src_ap, start=True, stop=True)
                t1 = mp_.tile([128, 512], bf16, tag="t1")
                nc.scalar.activation(t1[:, :cw], ps1[:, :cw], RELU, bias=b1)
                ps2 = ppm.tile([128, 512], f32, tag="psmlp")
                nc.tensor.matmul(out=ps2[:, :cw], lhsT=w2, rhs=t1[:, :cw], start=True, stop=True)
                nc.scalar.activation(dst_ap, ps2[:, :cw], RELU, bias=b2)

            def store_table(tblap, h_tile, r0, cw):
                for j in range(cw // 128):
                    pstp = ppt.tile([128, 128], bf16, tag="pstp")
                    nc.tensor.transpose(out=pstp[:], in_=h_tile[:, j * 128:(j + 1) * 128], identity=ident[:])
                    ht = tp.tile([128, 128], bf16, tag="ht")
                    nc.vector.tensor_copy(ht[:], pstp[:])
                    nc.sync.dma_start(tblap[r0 + j * 128: r0 + (j + 1) * 128, :], ht[:])

            def scatter_pass(tbl, gidx, dloc, nw, T, dst_sb, evac_bf16=True):
                tile_idx = 0
                cur = [None, None]

                def need(k):
                    g = gp.tile([128, 8, 128], bf16, tag="g")
                    nc.gpsimd.dma_gather(
                        g[:], tbl[:], gidx[:, k * 64:(k + 1) * 64],
                        num_idxs=GATHER_CHUNK, num_idxs_reg=GATHER_CHUNK,
                        elem_size=128, queue_num=k % 4,
                    )
                    oh = ohp.tile([128, 8, 128], bf16, tag="oh")
                    nc.vector.tensor_tensor(
                        out=oh[:],
                        in0=iota8[:].rearrange("p (a j) -> p a j", j=128),
                        in1=dloc[:, k * 8:(k + 1) * 8].to_broadcast([128, 8, 128]),
                        op=EQ,
                    )
                    cur[0], cur[1] = g, oh

                for w in range(nw):
                    tw = T[w]
                    if tw == 0:
                        continue
                    psw = pp.tile([128, 128], f32, tag="psw")
                    for t in range(tw):
                        k, j = divmod(tile_idx, 8)
                        if j == 0:
                            need(k)
                        nc.tensor.matmul(
                            out=psw[:], lhsT=cur[0][:, j, :], rhs=cur[1][:, j, :],
                            start=(t == 0), stop=(t == tw - 1),
                            skip_group_check=True,
                        )
                        tile_idx += 1
                    nc.scalar.activation(dst_sb[:, w * 128:(w + 1) * 128], psw[:], COPY)

            def chunks(total):
                c0 = 0
                while c0 < total:
                    cw = min(512, total - c0)
                    yield c0, cw
                    c0 += cw

            for l in range(L):
                # A: node enc MLP -> tbl_h
                for c0, cw in chunks(NSP):
                    h = mp_.tile([128, 512], bf16, tag="h")
                    mlp_chunk(h[:, :cw], node_fm[:, c0:c0 + cw],
                              W("ve_enc", l, 1), B("ve_enc", l, 1),
                              W("ve_enc", l, 2), B("ve_enc", l, 2), cw)
                    store_table(tbl_h, h, c0, cw)

                # B: V2E gather+scatter into edge_acc
                nc.vector.memset(edge_acc[:], 0.0)
                scatter_pass(tbl_h, gidx1, dloc1, NW1, T1, edge_acc)

                # C: AllReduce edge partials
                cc_in = dram.tile([128, MP], bf16, tag="cci")
                cc_out = dram.tile([128, MP], bf16, tag="cco")
                nc.gpsimd.dma_start(cc_in[:], edge_acc[:])
                nc.gpsimd.collective_compute(
                    "AllReduce", mybir.AluOpType.add,
                    replica_groups=[list(range(NCORES))],
                    ins=[cc_in[:].opt()], outs=[cc_out[:].opt()],
                )

                # D: edge MLPs (ve_dec then ev_enc) -> tbl_e
                for c0, cw in chunks(MP):
                    ce = mp_.tile([128, 512], bf16, tag="ce")
                    nc.sync.dma_start(ce[:, :cw], cc_out[:, c0:c0 + cw])
                    ed = mp_.tile([128, 512], bf16, tag="ed")
                    mlp_chunk(ed[:, :cw], ce[:, :cw],
                              W("ve_dec", l, 1), B("ve_dec", l, 1),
                              W("ve_dec", l, 2), B("ve_dec", l, 2), cw)
                    ee = mp_.tile([128, 512], bf16, tag="ee")
                    mlp_chunk(ee[:, :cw], ed[:, :cw],
                              W("ev_enc", l, 1), B("ev_enc", l, 1),
                              W("ev_enc", l, 2), B("ev_enc", l, 2), cw)
                    store_table(tbl_e, ee, c0, cw)

                # E: E2V gather+scatter into node_fm
                nc.vector.memset(node_fm[:], 0.0)
                scatter_pass(tbl_e, gidx2, nloc2, NW2, T2, node_fm)

                # F: node dec MLP in place
                for c0, cw in chunks(NSP):
                    mlp_chunk(node_fm[:, c0:c0 + cw], node_fm[:, c0:c0 + cw],
                              W("ev_dec", l, 1), B("ev_dec", l, 1),
                              W("ev_dec", l, 2), B("ev_dec", l, 2), cw)

            # ---- readout ----
            ps_r = prp.tile([64, 128], f32, tag="psr")
            for w in range(NW2):
                pstp = ppt.tile([128, 128], bf16, tag="pstp")
                nc.tensor.transpose(out=pstp[:], in_=node_fm[:, w * 128:(w + 1) * 128], identity=ident[:])
                xnm = tp.tile([128, 128], bf16, tag="xnm")
                nc.vector.tensor_copy(xnm[:], pstp[:])
                nc.tensor.matmul(out=ps_r[:], lhsT=gmat[:, w * 64:(w + 1) * 64],
                                 rhs=xnm[:], start=(w == 0), stop=(w == NW2 - 1),
                                 skip_group_check=True)
            rd_sb = mp_.tile([64, 128], f32, tag="rd")
            nc.vector.tensor_copy(rd_sb[:], ps_r[:])
            rd_in = dram.tile([64, 128], f32, tag="rdi")
            rd_out = dram.tile([64, 128], f32, tag="rdo")
            nc.gpsimd.dma_start(rd_in[:], rd_sb[:])
            nc.gpsimd.collective_compute(
                "AllReduce", mybir.AluOpType.add,
                replica_groups=[list(range(NCORES))],
                ins=[rd_in[:].opt()], outs=[rd_out[:].opt()],
            )
            rsum = mp_.tile([64, 128], bf16, tag="rs")
            nc.gpsimd.dma_start(rsum[:], rd_out[:])

            # classifier: transpose r -> [128, 64], mm1+relu, then
            # out[g, c] = sum_dh hc[dh, g] * W2c[dh, c]  (lhsT=hc, rhs=W2c)
            ps_t = ppt.tile([128, 64], bf16, tag="pstp")
            nc.tensor.transpose(out=ps_t[:], in_=rsum[:], identity=ident[:64, :64])
            rT = tp.tile([128, 64], bf16, tag="rT")
            nc.vector.tensor_copy(rT[:], ps_t[:])
            ps_c1 = ppm.tile([128, 64], f32, tag="psmlp")
            nc.tensor.matmul(out=ps_c1[:], lhsT=wts[:, 16 * 128:17 * 128], rhs=rT[:],
                             start=True, stop=True)
            hc = tp.tile([128, 64], bf16, tag="hc")
            nc.scalar.activation(hc[:], ps_c1[:], RELU, bias=bias[:, 16:17])
            ps_o = ppm.tile([64, 40], f32, tag="psmlp")
            nc.tensor.matmul(out=ps_o[:], lhsT=hc[:], rhs=wts[:, 17 * 128:17 * 128 + 40],
                             start=True, stop=True)
            out_sb = tp.tile([64, 40], f32, tag="osb")
            nc.vector.tensor_tensor(out=out_sb[:], in0=ps_o[:],
                                    in1=b2row[:, :40],
                                    op=mybir.AluOpType.add)
            nc.sync.dma_start(out[:], out_sb[:])

    nc.compile()
    return nc


_CACHE = {}


def _get_nc(cfg):
    key = (cfg["NSP"], cfg["MP"], cfg["NT1"], cfg["NT2"], tuple(cfg["T1"]),
           tuple(cfg["T2"]), cfg["G"], cfg["L"])
    if key not in _CACHE:
        _CACHE[key] = _build(cfg)
    return _CACHE[key]


def kernel(**inputs):
    X = np.asarray(inputs["X"])
    N, _ = X.shape
    E = np.asarray(inputs["v2e_src"]).shape[0]
    M = 20000 if N == 100000 else int(np.asarray(inputs["v2e_dst"]).max()) + 1
    G = 64 if N == 100000 else int(np.asarray(inputs["all_batch"]).max()) + 1
    L = np.asarray(inputs["ve_enc_W1"]).shape[0]
    if N == 100000:
        M, G = 20000, 64
    in_maps, cfg = _preprocess(inputs, N, M, E, G, L)
    nc = _get_nc(cfg)
    res = run_bass_kernel_spmd(nc, in_maps, core_ids=list(range(NCORES)))
    return np.asarray(res.results[0]["out"], np.float32)
